# revision 33
# baseline (speedup 1.0000x reference)
"""BitSPPF kernel for Trainium2 (8 NeuronCores, data-parallel over batch).

Pipeline per core (4 images):
  cv1 (1x1 ternary-quantized conv, bf16 matmuls) -> BN+SiLU (ACT engine)
  -> 3x chained 5x5 maxpool (separable max trees on DVE, bf16, processed
  two channel-tiles per instruction) -> centered fp8 re-encode (ACT)
  -> cv2 in fp8 DoubleRow (2x PE throughput) -> BN+SiLU -> DRAM.

cv2's fp8 precision is recovered by per-channel mean centering: each of
[h, y1, y2, y3] is stored as (v - c) in fp8 with the per-channel c folded
back via a host-computed bias correction W@c added to cv2's BN bias. The
subtraction rides for free on the ACT engine's Identity(x + bias) dtype
conversion. Calibration c comes from a 2-image host-side pass in _prep.

Schedule per image (software pipeline, lag 2): cv1(b), then the six pool
chain levels of b (level-major, A/B channel-tile pairs) emitted
interleaved with cv2(b-2) half-units and the fp8 encodes in readiness
order, so the in-order ACT queue never backs up into PE's psum ring and
the DVE stays saturated. The final image's cv2 streams its
y3-independent k-pairs ahead while the last pool chain finishes.
"""

import os
import sys

for _p in ("/opt/trn_rl_repo",):
    if _p not in sys.path and os.path.isdir(_p):
        sys.path.insert(0, _p)

import numpy as np
import ml_dtypes

import concourse.bass as bass
import concourse.tile as tile
from concourse import bacc, mybir

BF16 = mybir.dt.bfloat16
F32 = mybir.dt.float32
FP8 = mybir.dt.float8e4
NPBF16 = ml_dtypes.bfloat16
NPFP8 = ml_dtypes.float8_e4m3

# Problem shapes (hardcoded per spec)
B, C1, H, W = 32, 1024, 40, 40
HID, C2 = 512, 1024
S = H * W  # 1600
N_CORES = 8
BL = B // N_CORES  # images per core

NEG = -3.0e38  # effectively -inf for maxpool padding, finite in bf16

EPS = 1e-8
BN_EPS = 1e-5

DR = mybir.MatmulPerfMode.DoubleRow


def _pools_chain(nc, P, HX, M2, Pout, padded_out):
    """One 5x5 stride-1 pad-2 maxpool over a channel-tile PAIR: P -> Pout.

    P: [128, 2, 40, 44] bf16, data in cols 2..41, cols {0,1,42,43} = NEG.
    HX: [128, 2, 44, 40] scratch; rows {0,1,42,43} pre-set to NEG.
    M2: [128, 2, 43, 44] scratch.
    Pout: [128, 2, 40, 44] (padded_out) or [128, 2, 40, 40].
    """
    nc.vector.tensor_max(M2[:, :, 0:40, 0:43], P[:, :, :, 0:43], P[:, :, :, 1:44])
    nc.vector.tensor_max(HX[:, :, 2:42, :], M2[:, :, 0:40, 0:40], M2[:, :, 0:40, 2:42])
    nc.vector.tensor_max(HX[:, :, 2:42, :], HX[:, :, 2:42, :], P[:, :, :, 4:44])
    nc.vector.tensor_max(M2[:, :, 0:43, 0:40], HX[:, :, 0:43, :], HX[:, :, 1:44, :])
    if padded_out:
        ov = Pout[:, :, :, 2:42]
    else:
        ov = Pout[:, :, :, :]
    nc.vector.tensor_max(ov, M2[:, :, 0:40, 0:40], M2[:, :, 2:42, 0:40])
    nc.vector.tensor_max(ov, ov, HX[:, :, 4:44, :])


def _build_nc(bl=BL):
    nc = bacc.Bacc(trn_type="TRN2", debug=False)

    xq_d = nc.dram_tensor("xq", [bl, C1, S], BF16, kind="ExternalInput")
    w1t_d = nc.dram_tensor("w1t", [C1, HID], BF16, kind="ExternalInput")
    w2t_d = nc.dram_tensor("w2t", [4 * HID, C2], FP8, kind="ExternalInput")
    sc1_d = nc.dram_tensor("sc1", [HID], F32, kind="ExternalInput")
    bi1_d = nc.dram_tensor("bi1", [HID], F32, kind="ExternalInput")
    sc2_d = nc.dram_tensor("sc2", [C2], F32, kind="ExternalInput")
    bi2_d = nc.dram_tensor("bi2", [C2], F32, kind="ExternalInput")
    cng_d = nc.dram_tensor("cng", [16 * 128], F32, kind="ExternalInput")
    out_d = nc.dram_tensor("out", [bl, C2, S], F32, kind="ExternalOutput")

    KT1 = C1 // 128       # 8 k-tiles for cv1
    MT1 = HID // 128      # 4 m-tiles (= pool channel tiles)
    KT2 = 4 * HID // 128  # 16 k-subtiles for cv2
    KP2 = KT2 // 2        # 8 fp8 DoubleRow pairs
    MT2 = C2 // 128       # 8 m-tiles for cv2
    NQ = 4                # spatial quarters of 400 cols (10 rows of 40)
    QW = S // NQ          # 400

    xv = xq_d.ap().rearrange("b (kt p) s -> b p kt s", p=128)
    ov = out_d.ap().rearrange("b (mt p) s -> b p mt s", p=128)

    # CoreSim doesn't implement Silu; allow substituting Sigmoid for
    # wiring-validation sim runs (numerics then differ by design).
    if os.environ.get("BITSPPF_SIM_ACT") == "sigmoid":
        silu = mybir.ActivationFunctionType.Sigmoid
    else:
        silu = mybir.ActivationFunctionType.Silu
    ident = mybir.ActivationFunctionType.Identity

    with tile.TileContext(nc) as tc:
        with (
            tc.tile_pool(name="const", bufs=1) as const,
            tc.tile_pool(name="xin", bufs=4) as xin,
            tc.tile_pool(name="pbuf0", bufs=2) as pbuf0,
            tc.tile_pool(name="plad", bufs=4) as plad,
            tc.tile_pool(name="v8p", bufs=3) as v8p,
            tc.tile_pool(name="work", bufs=1) as work,
            tc.tile_pool(name="osb", bufs=3) as osb,
            tc.tile_pool(name="ps1", bufs=2, space="PSUM") as ps1p,
            tc.tile_pool(name="ps2", bufs=3, space="PSUM") as ps2p,
        ):
            # Pre-warm the ACT engine's Silu spline tables (~2.7us load)
            # during the initial DMA window instead of at the first real
            # activation.
            warm = const.tile([128, 2], F32)
            nc.vector.memset(warm, 0.0)
            nc.scalar.activation(out=warm, in_=warm, func=silu)

            w1_sb = const.tile([128, KT1, HID], BF16)
            nc.sync.dma_start(w1_sb, w1t_d.ap().rearrange("(kt p) m -> p kt m", p=128))
            sc1_sb = const.tile([128, MT1], F32)
            nc.sync.dma_start(sc1_sb, sc1_d.ap().rearrange("(t p) -> p t", p=128))
            bi1_sb = const.tile([128, MT1], F32)
            nc.sync.dma_start(bi1_sb, bi1_d.ap().rearrange("(t p) -> p t", p=128))

            def load_small_consts():
                sc2_sb = const.tile([128, MT2], F32)
                nc.sync.dma_start(sc2_sb, sc2_d.ap().rearrange("(t p) -> p t", p=128))
                bi2_sb = const.tile([128, MT2], F32)
                nc.sync.dma_start(bi2_sb, bi2_d.ap().rearrange("(t p) -> p t", p=128))
                cng_sb = const.tile([128, 16], F32)
                nc.sync.dma_start(cng_sb, cng_d.ap().rearrange("(t p) -> p t", p=128))
                return sc2_sb, bi2_sb, cng_sb

            def load_w2():
                w2_sb = const.tile([128, KT2, C2], FP8)
                nc.sync.dma_start(
                    w2_sb, w2t_d.ap().rearrange("(kt p) m -> p kt m", p=128)
                )
                return w2_sb

            # PE HAM warm-up: keep the PE activity window busy from the
            # moment the (tiny, early-landing) sc1 constants arrive until the
            # first real matmul, so the clock gate is already at 8/8 when it
            # issues.
            wps = ps1p.tile([128, 512], F32, tag="ps1")
            for _i in range(80):
                nc.tensor.matmul(
                    wps[0:4, 0:4], sc1_sb, sc1_sb, start=True, stop=True,
                )
            for _i in range(30):
                nc.tensor.matmul(
                    wps[:, 0:32], w1_sb[:, 0, 0:128], w1_sb[:, 0, 0:32],
                    start=True, stop=True,
                )

            pimg = {}  # b -> [P0 pair list, V8]

            def emit_cv1(b, mts=(0, 1, 2, 3), xss=None):
                """cv1 over the given mts + their h8 encodes. When ``xss``
                is given, the quarter tiles were pre-loaded by the caller
                (image 0 holds all 4 so pools can start at half-cv1)."""
                if mts[0] == 0:
                    v8 = v8p.tile([128, KT2, 40, 40], FP8, tag="V8")
                    pimg[b] = [[], v8]
                P0s, v8 = pimg[b]
                for _pr in range(len(mts) // 2):
                    P0 = pbuf0.tile([128, 2, 40, 44], BF16, tag="P0")
                    nc.gpsimd.memset(P0[:, :, :, 0:2], NEG)
                    nc.gpsimd.memset(P0[:, :, :, 42:44], NEG)
                    P0s.append(P0)
                for q in range(NQ):
                    if xss is None:
                        xs = xin.tile([128, KT1, QW], BF16, tag="x")
                        nc.sync.dma_start(xs, xv[b][:, :, q * QW:(q + 1) * QW])
                    else:
                        xs = xss[q]
                    for mt in mts:
                        ps = ps1p.tile([128, 512], F32, tag="ps1")
                        for kt in range(KT1):
                            nc.tensor.matmul(
                                ps[:, :QW],
                                w1_sb[:, kt, mt * 128:(mt + 1) * 128],
                                xs[:, kt, :],
                                start=(kt == 0),
                                stop=(kt == KT1 - 1),
                            )
                        nc.scalar.activation(
                            out=P0s[mt // 2][:, mt % 2, q * 10:(q + 1) * 10, 2:42],
                            in_=ps[:, :QW],
                            func=silu,
                            bias=bi1_sb[:, mt:mt + 1],
                            scale=sc1_sb[:, mt:mt + 1],
                        )
                for ct in mts:
                    nc.scalar.activation(
                        out=v8[:, ct], in_=P0s[ct // 2][:, ct % 2, :, 2:42],
                        func=ident, bias=cng_sb[:, ct:ct + 1],
                    )

            def emit_chain_level(b, pr, src, padded_out):
                """One maxpool level for ct pair pr; returns the pair tile."""
                HX = work.tile([128, 2, 44, 40], BF16, tag="HX", bufs=2)
                M2 = work.tile([128, 2, 43, 44], BF16, tag="M2", bufs=2)
                nc.gpsimd.memset(HX[:, :, 0:2, :], NEG)
                nc.gpsimd.memset(HX[:, :, 42:44, :], NEG)
                po = plad.tile([128, 2, 40, 44], BF16, tag="PL")
                if padded_out:
                    nc.gpsimd.memset(po[:, :, :, 0:2], NEG)
                    nc.gpsimd.memset(po[:, :, :, 42:44], NEG)
                    _pools_chain(nc, src, HX, M2, po, True)
                else:
                    _pools_chain(nc, src, HX, M2, po[:, :, :, 0:40], False)
                return po

            def emit_enc(b, pr, lvl, pl):
                """fp8 re-encode of pool level lvl (1..3) for ct pair pr."""
                v8 = pimg[b][1]
                src = pl[:, :, :, 2:42] if lvl < 3 else pl[:, :, :, 0:40]
                for i in range(2):
                    ct = 2 * pr + i
                    j = lvl * MT1 + ct
                    nc.scalar.activation(
                        out=v8[:, j], in_=src[:, i], func=ident,
                        bias=cng_sb[:, j:j + 1],
                    )

            def emit_cv2_half_mm(b, mt2, h, psU, kps):
                v8 = pimg[b][1]
                for kp in kps:
                    lhs = w2_sb[:, 2 * kp:2 * kp + 2, mt2 * 128:(mt2 + 1) * 128]
                    st = kp == 0
                    sp = kp == KP2 - 1
                    for j in range(2):
                        nt = 2 * h + j
                        nc.tensor.matmul(
                            psU[:, j, :QW], lhs,
                            v8[:, 2 * kp:2 * kp + 2, nt * 10:(nt + 1) * 10, :],
                            start=st, stop=sp, perf_mode=DR,
                        )

            def emit_cv2_half_out(b, mt2, h, psU):
                for j in range(2):
                    nt = 2 * h + j
                    oo = osb.tile([128, QW], F32, tag="o")
                    nc.scalar.activation(
                        out=oo, in_=psU[:, j, :QW], func=silu,
                        bias=bi2_sb[:, mt2:mt2 + 1],
                        scale=sc2_sb[:, mt2:mt2 + 1],
                    )
                    nc.sync.dma_start(ov[b][:, mt2, nt * QW:(nt + 1) * QW], oo)

            def emit_cv2_half(b, mt2, h):
                """cv2 for (mt2, spatial half h): one 2-bank psum tile."""
                psU = ps2p.tile([128, 2, 512], F32, tag="ps2")
                emit_cv2_half_mm(b, mt2, h, psU, range(KP2))
                emit_cv2_half_out(b, mt2, h, psU)

            def emit_image(b, bcv2):
                """One pipeline iteration: cv1(b), pool levels of b emitted
                level-major (A then B per level) with the fp8 encodes and
                cv2(bcv2) half-units interleaved so the in-order ACT queue
                tracks readiness on all fronts."""
                if b == 0:
                    # image 0: pre-load all 4 x quarters and run cv1 in two
                    # mt-pair passes over the held tiles, so the first pool
                    # chains start at half-cv1 (smaller pipeline fill)
                    xss = []
                    for q in range(NQ):
                        xs = xin.tile([128, KT1, QW], BF16, tag="x")
                        nc.sync.dma_start(xs, xv[b][:, :, q * QW:(q + 1) * QW])
                        xss.append(xs)
                    emit_cv1(b, (0, 1), xss)
                    p1A = emit_chain_level(b, 0, pimg[b][0][0], True)
                    emit_cv1(b, (2, 3), xss)
                else:
                    emit_cv1(b)
                    p1A = emit_chain_level(b, 0, pimg[b][0][0], True)

                def cv2h(k):
                    if bcv2 is not None:
                        emit_cv2_half(bcv2, k // 2, k % 2)

                p1B = emit_chain_level(b, 1, pimg[b][0][1], True)
                cv2h(0); cv2h(1); cv2h(2)
                emit_enc(b, 0, 1, p1A)
                cv2h(3)
                p2A = emit_chain_level(b, 0, p1A, True)
                cv2h(4); cv2h(5)
                emit_enc(b, 1, 1, p1B)
                cv2h(6)
                p2B = emit_chain_level(b, 1, p1B, True)
                cv2h(7); cv2h(8)
                emit_enc(b, 0, 2, p2A)
                cv2h(9); cv2h(10)
                p3A = emit_chain_level(b, 0, p2A, False)
                cv2h(11)
                emit_enc(b, 1, 2, p2B)
                cv2h(12); cv2h(13)
                p3B = emit_chain_level(b, 1, p2B, False)
                cv2h(14); cv2h(15)
                emit_enc(b, 0, 3, p3A)
                emit_enc(b, 1, 3, p3B)

            lag = 2 if bl > 2 else 1
            sc2_sb, bi2_sb, cng_sb = load_small_consts()
            w2_sb = None
            for b in range(bl):
                emit_image(b, b - lag if b >= lag else None)
                if b == 0:
                    # the 2MB w2 load rides the DMA queues behind image 0's
                    # x tiles; it's only needed by cv2 two iterations later
                    w2_sb = load_w2()
            # Tail: cv2 of the last `lag` images. For the final image, stream
            # the y3-independent kp 0..5 of each half ahead (3 psum tiles in
            # flight) so only the kp6/7 accumulations serialize behind the
            # last pool chain.
            for b in range(max(0, bl - lag), bl - 1):
                for k in range(2 * MT2):
                    emit_cv2_half(b, k // 2, k % 2)
            b = bl - 1
            pend = []  # (k, psU) with kp0-5 streamed
            for k in range(2 * MT2):
                psU = ps2p.tile([128, 2, 512], F32, tag="ps2")
                emit_cv2_half_mm(b, k // 2, k % 2, psU, range(6))
                pend.append((k, psU))
                if len(pend) == 3 or k == 2 * MT2 - 1:
                    while pend:
                        kk, ps_ = pend.pop(0)
                        emit_cv2_half_mm(b, kk // 2, kk % 2, ps_, (6, 7))
                        emit_cv2_half_out(b, kk // 2, kk % 2, ps_)
                        if k < 2 * MT2 - 1:
                            break

    nc.compile()
    return nc


_NC_CACHE = {}


def _get_nc(bl=BL):
    if bl not in _NC_CACHE:
        _NC_CACHE[bl] = _build_nc(bl)
    return _NC_CACHE[bl]


def _maxpool5_np(x):
    """x: [C, H, W] f32 -> 5x5 stride-1 pad-2 maxpool."""
    C, HH, WW = x.shape
    xp = np.full((C, HH + 4, WW + 4), -np.inf, np.float32)
    xp[:, 2:-2, 2:-2] = x
    out = np.full((C, HH, WW), -np.inf, np.float32)
    for dy in range(5):
        for dx in range(5):
            np.maximum(out, xp[:, dy:dy + HH, dx:dx + WW], out=out)
    return out


def _prep(inputs):
    """Host-side: quantize weights to ternary, fold BitNet scale + BN into
    per-channel (scale, bias), build the fp8 cv2 weights and the
    per-channel centering constants + bias correction."""
    x = np.asarray(inputs["x"], dtype=np.float32)
    w1 = np.asarray(inputs["w1"], dtype=np.float32)
    w2 = np.asarray(inputs["w2"], dtype=np.float32)
    g1 = np.asarray(inputs["g1"], dtype=np.float32)
    b1 = np.asarray(inputs["b1"], dtype=np.float32)
    m1 = np.asarray(inputs["m1"], dtype=np.float32)
    v1 = np.asarray(inputs["v1"], dtype=np.float32)
    g2 = np.asarray(inputs["g2"], dtype=np.float32)
    b2 = np.asarray(inputs["b2"], dtype=np.float32)
    m2 = np.asarray(inputs["m2"], dtype=np.float32)
    v2 = np.asarray(inputs["v2"], dtype=np.float32)

    def fold(w, g, b, m, v):
        s = np.float32(max(np.median(np.abs(w)), EPS))
        t = np.clip(np.round(w / s), -1.0, 1.0).astype(np.float32)
        inv = g / np.sqrt(v + BN_EPS)
        scale = (s * inv).astype(np.float32)
        bias = (b - m * inv).astype(np.float32)
        return t, scale, bias

    t1, sc1, bi1 = fold(w1, g1, b1, m1, v1)
    t2, sc2, bi2 = fold(w2, g2, b2, m2, v2)

    Wa, Wb, Wc, Wd = (t2[:, i * HID:(i + 1) * HID] for i in range(4))
    w2p = t2

    # Calibration: per-channel means of h, y1, y2, y3 from 2 images.
    nb = x.shape[0]
    cal_imgs = [0, nb // 2] if nb > 1 else [0]
    cals = []
    for bi_ in cal_imgs:
        xb = x[bi_].reshape(C1, S).astype(NPBF16).astype(np.float32)
        ps1 = t1 @ xb
        pre = sc1[:, None] * ps1 + bi1[:, None]
        h = (pre / (1.0 + np.exp(-pre))).astype(NPBF16).astype(np.float32)
        y1 = _maxpool5_np(h.reshape(HID, H, W))
        y2 = _maxpool5_np(y1)
        y3 = _maxpool5_np(y2)
        cals.append([h.mean(axis=1), y1.reshape(HID, S).mean(axis=1),
                     y2.reshape(HID, S).mean(axis=1),
                     y3.reshape(HID, S).mean(axis=1)])
    ch, cy1, cy2_, cy3_ = (
        np.mean([c[i] for c in cals], axis=0).astype(np.float32)
        for i in range(4)
    )

    # bias correction: cv2 sees centered blocks, so add back W@c
    corr = Wa @ ch + Wb @ cy1 + Wc @ cy2_ + Wd @ cy3_
    bi2e = (bi2 + sc2 * corr).astype(np.float32)

    # negated centering constants, packed per V8 k-subtile [16*128]
    cneg = np.concatenate([-ch, -cy1, -cy2_, -cy3_]).astype(np.float32)

    w1t = np.ascontiguousarray(t1.T).astype(NPBF16)
    w2t = np.ascontiguousarray(w2p.T).astype(NPFP8)

    xq = x.reshape(x.shape[0], C1, S).astype(NPBF16)
    shared = dict(w1t=w1t, w2t=w2t, sc1=sc1, bi1=bi1, sc2=sc2, bi2=bi2e,
                  cng=cneg)
    in_maps = []
    for d in range(N_CORES):
        m = dict(shared)
        m["xq"] = np.ascontiguousarray(xq[d * BL:(d + 1) * BL])
        in_maps.append(m)
    return in_maps


def _install_ntff_hook():
    """The agent image's antenv lacks axon_hooks; synthesize it so
    run_bass_kernel_spmd(trace=True) can capture NTFF profiles via the
    axon .so's C ABI (same mechanism trn_boot would install)."""
    import types

    try:
        import antenv.axon_hooks  # noqa: F401

        return
    except ImportError:
        pass
    try:
        import antenv

        bootdir = "/root/.axon_site/trn_agent_boot"
        if bootdir not in sys.path and os.path.isdir(bootdir):
            sys.path.insert(0, bootdir)
        import trn_boot

        hook = trn_boot._ntff_profile_via_ctypes("/opt/axon/libaxon_pjrt.so")
        mod = types.ModuleType("antenv.axon_hooks")
        state = {"h": hook}
        mod.get_axon_ntff_profile_hook = lambda: state["h"]
        mod.set_axon_ntff_profile_hook = lambda h: state.update(h=h)
        sys.modules["antenv.axon_hooks"] = mod
        antenv.axon_hooks = mod
    except Exception as e:  # profiling is best-effort; execution still works
        print(f"ntff hook install failed: {e}", file=sys.stderr)


def _run(inputs, trace=False):
    from concourse import bass_utils

    if trace:
        _install_ntff_hook()
    nc = _get_nc()
    in_maps = _prep(inputs)
    import time

    res = None
    for attempt, delay in ((0, 5), (1, 20), (2, 0)):
        try:
            res = bass_utils.run_bass_kernel_spmd(
                nc, in_maps, core_ids=list(range(N_CORES)), trace=trace,
            )
            break
        except Exception as e:  # transient device errors happen; back off
            if attempt == 2:
                raise
            print(
                f"run_bass_kernel_spmd failed ({type(e).__name__}); "
                f"retrying in {delay}s",
                file=sys.stderr,
            )
            time.sleep(delay)
    assert res is not None
    outs = [res.results[d]["out"] for d in range(N_CORES)]
    full = np.concatenate(outs, axis=0).reshape(B, C2, H, W).astype(np.float32)
    return full, res


def kernel(**inputs):
    full, _ = _run(inputs, trace=False)
    return full


def run_traced(**inputs):
    full, res = _run(inputs, trace=True)
    return full, res.exec_time_ns
